# revision 30
# baseline (speedup 1.0000x reference)
"""Trainium2 Bass kernel for nn_BandwidthConstrainedComm.

GNN message passing: per batch element, N=256 agents each generate a
message (MLP -> compress -> decompress), compute pairwise bilinear
relevance scores, pick top-K=8 senders (softmax gated), aggregate their
messages, and run a receiver MLP over [obs, agg].

Sharding: pure data parallel over batch B=128 -> 16 per core x 8 cores.

Design notes (v4 - 5-stage pipeline + fp8 DoubleRow front-end):
  - obs uploaded twice: bf16 [d,2(dc),b,n] for the receiver MLP (fp8
    there breaks tolerance) and fp8-e4m3 [d_lo,2(d_hi),b,n] for the
    message/score path (verified: max-err unchanged, the bf16 receiver
    path dominates).
  - h-MLP, bilinear tmp and scores run as fp8 DoubleRow matmuls
    (K=256 per instruction, ~2x fewer PE instructions, 1.44x rate).
    tmpT is cast PSUM->fp8 so scores' stationary operand is fp8 too.
  - W2@Wc@Wd fused into one [H1, MSG] matrix on the host; message bias
    (+ br1) folded into the receiver matmul via a ones-row in aggT and
    an extra host-precomputed row in Wr1c; br2 added on the host.
  - top-8 via DVE Max8; den=sum(top8) one reduce; msk=(E>=t8)*rden on
    DVE (2-scalar tensor_scalar); U = msk*E on GPSIMD tensor_tensor
    (the only fast Pool op); gate transpose Gt = U.T @ I by PE.
  - 5-stage pipeline: pre(v) | gate(v-1) | Gt+agg(v-2) | l1(v-3) |
    l2+out(v-4), with per-engine queue orders chosen so every
    cross-engine dependency has >= half an iteration of slack.
  - engine budget per pair: PE ~4us, scalar relu_h/out/msn/Gt-cast/
    relu_r/exp ~5.5us, vector tmpT/Max8/den/recip/msk/aggT ~5.3us,
    gpsimd 4 mults ~2.7us.
  - PSUM 16KB/partition exactly: g[2K]=hT/msn, a[4K]=tmp/r, b[4K]=o/s,
    c[4K]=Gt, e[2K]=aggT.
  - output written as bf16 [D, bpc, N] in one DMA, un-transposed and
    f32-cast on the host.
"""

import sys

sys.path.insert(0, "/opt/trn_rl_repo")

import numpy as np

# problem dims (hardcoded per contract)
B, N, D = 128, 256, 256
MSG, CD, K = 64, 32, 8
H1, H2 = 128, 256
NCORES = 8
BPC = B // NCORES  # batches per core

_CACHE = {}


def build_program(bpc=BPC, passes=1):
    import concourse.bacc as bacc
    import concourse.mybir as mybir
    import concourse.tile as tile
    from concourse.masks import make_identity
    from contextlib import ExitStack

    dt = mybir.dt
    f32, bf16, f8 = dt.float32, dt.bfloat16, dt.float8e4
    AF = mybir.ActivationFunctionType
    OP = mybir.AluOpType
    DR = mybir.MatmulPerfMode.DoubleRow

    assert bpc % 2 == 0
    npairs = bpc // 2

    nc = bacc.Bacc("TRN2", target_bir_lowering=False, debug=False,
                   num_devices=NCORES)

    obsT_d = nc.dram_tensor("obsT", [D, bpc, N], bf16, kind="ExternalInput")
    obs8_d = nc.dram_tensor("obs8", [128, 2, bpc, N], f8,
                            kind="ExternalInput")
    W1_d = nc.dram_tensor("W1", [128, 2, H1], f8, kind="ExternalInput")
    Wf_d = nc.dram_tensor("Wf", [H1, MSG], bf16, kind="ExternalInput")
    Wbil_d = nc.dram_tensor("Wbil", [128, 2, D], f8, kind="ExternalInput")
    Wr1a_d = nc.dram_tensor("Wr1a", [D, H2], bf16, kind="ExternalInput")
    Wr1c_d = nc.dram_tensor("Wr1c", [MSG + 1, H2], bf16,
                            kind="ExternalInput")
    Wr2_d = nc.dram_tensor("Wr2", [H2, D], bf16, kind="ExternalInput")
    b1_d = nc.dram_tensor("b1", [H1], f32, kind="ExternalInput")
    out_d = nc.dram_tensor("out", [D, bpc, N], bf16, kind="ExternalOutput")

    with tile.TileContext(nc) as tc, ExitStack() as ctx:
        wp = ctx.enter_context(tc.tile_pool(name="wp", bufs=1))
        dp = ctx.enter_context(tc.tile_pool(name="dp", bufs=5))
        sp = ctx.enter_context(tc.tile_pool(name="sp", bufs=3))
        pp = ctx.enter_context(tc.tile_pool(name="pp", bufs=1, space="PSUM"))

        # ---------------- one-time setup ----------------
        ident = wp.tile([128, 128], f32)
        make_identity(nc, ident[:])
        ident_b = wp.tile([128, 128], bf16)
        nc.vector.tensor_copy(ident_b[:], ident[:])

        def loadw(dram_ap, shape, name, dtype=bf16, eng=nc.scalar):
            t = wp.tile(shape, dtype, name=name)
            eng.dma_start(t[:], dram_ap)
            return t

        W1_8 = loadw(W1_d[:], [128, 2, H1], "W1", f8)
        Wbil_8 = loadw(Wbil_d[:], [128, 2, D], "Wbil", f8, nc.gpsimd)
        Wf_b = loadw(Wf_d[:], [H1, MSG], "Wf")
        Wr1_r0 = loadw(Wr1a_d[0:128, :], [128, H2], "Wr1a", bf16, nc.gpsimd)
        Wr1_r1 = loadw(Wr1a_d[128:256, :], [128, H2], "Wr1b")
        Wr1c_b = loadw(Wr1c_d[:], [MSG + 1, H2], "Wr1c", bf16, nc.gpsimd)
        Wr2_r0 = loadw(Wr2_d[0:128, :], [128, D], "Wr2a")
        Wr2_r1 = loadw(Wr2_d[128:256, :], [128, D], "Wr2b", bf16, nc.gpsimd)

        b1_sb = wp.tile([H1, 1], f32, name="b1s")
        nc.scalar.dma_start(
            b1_sb[:], b1_d[:].rearrange("(p o) -> p o", o=1))

        # persistent aggT tiles with a constant ones-row (row MSG) for
        # the folded message bias (+ br1)
        aggT_tiles = []
        for i in range(2):
            t = wp.tile([MSG + 1, 2, N], bf16, name=f"aggTp{i}")
            nc.vector.memset(t[MSG:MSG + 1, :, :], 1.0)
            aggT_tiles.append(t)

        obsT_v = obsT_d[:].rearrange("(c d) b n -> d c b n", c=2)
        out_v = out_d[:].rearrange("(c d) b n -> d c b n", c=2)

        # ---------------- pipeline stages ----------------
        state = {}

        def emit_od(p):
            o8 = dp.tile([128, 2, 2, N], f8, name="od8", tag="od8", bufs=3)
            nc.sync.dma_start(o8[:], obs8_d[:, :, 2 * p:2 * p + 2, :])
            ob = dp.tile([128, 2, 2, N], bf16, name="od", tag="od", bufs=5)
            nc.sync.dma_start(ob[:], obsT_v[:, :, 2 * p:2 * p + 2, :])
            state[("od", p)] = ob
            state[("od8", p)] = o8

        def emit_ht(p):
            o8 = state[("od8", p)]
            hT_ps = pp.tile([H1, 2 * N], f32, tag="g", bufs=1)
            nc.tensor.matmul(hT_ps[:], W1_8[:],
                             o8[:].rearrange("d c b n -> d c (b n)"),
                             start=True, stop=True, perf_mode=DR)
            hT_b = sp.tile([H1, 2 * N], bf16, name="hT_b", tag="hT",
                           bufs=3)
            nc.scalar.activation(hT_b[:], hT_ps[:], AF.Relu, bias=b1_sb[:])
            state[("hT", p)] = hT_b

        def emit_tmp(p):
            o8 = state[("od8", p)]
            rhs = o8[:].rearrange("d c b n -> d c (b n)")
            tmp_ps = pp.tile([128, 2, 2 * N], f32, tag="a", bufs=1)
            for ec in range(2):
                nc.tensor.matmul(tmp_ps[:, ec, :],
                                 Wbil_8[:, :, 128 * ec:128 * (ec + 1)],
                                 rhs, start=True, stop=True, perf_mode=DR)
            tmpT8 = sp.tile([128, 2, 2 * N], f8, name="tmpT8",
                            tag="tmpT", bufs=3)
            nc.vector.tensor_copy(
                tmpT8[:].rearrange("e c f -> e (c f)"),
                tmp_ps[:].rearrange("e c f -> e (c f)"))
            state[("tmpT", p)] = tmpT8

        def emit_msn(p):
            hT_b = state.pop(("hT", p))
            msn_ps = pp.tile([128, 4, MSG], f32, tag="g", bufs=1,
                             name="msn_ps")
            for q in range(4):
                nc.tensor.matmul(msn_ps[:, q, :],
                                 hT_b[:, 128 * q:128 * (q + 1)],
                                 Wf_b[:], start=True, stop=True)
            msgs_b = sp.tile([128, 4, MSG], bf16, name="msgs_b",
                             tag="msgs", bufs=4)
            nc.scalar.activation(
                msgs_b[:].rearrange("p q m -> p (q m)"),
                msn_ps[:].rearrange("p q m -> p (q m)"), AF.Copy)
            state[("msgs", p)] = msgs_b

        def emit_scores(p):
            o8 = state[("od8", p)]
            tmpT8 = state.pop(("tmpT", p))
            s_ps = pp.tile([128, 2, 2, N], f32, tag="b", bufs=1,
                           name="s_ps")
            for bi in range(2):
                boff = bi * N
                for ic in range(2):
                    ioff = boff + 128 * ic
                    nc.tensor.matmul(s_ps[:, bi, ic, :],
                                     tmpT8[:, :, ioff:ioff + 128],
                                     o8[:, :, bi, :],
                                     start=True, stop=True, perf_mode=DR)
            state[("s_ps", p)] = s_ps

        def emit_exp(p):
            s_ps = state.pop(("s_ps", p))
            E = sp.tile([128, 4, N], bf16, name="E", tag="E", bufs=5)
            nc.scalar.activation(
                E[:].rearrange("p c f -> p (c f)"),
                s_ps[:].rearrange("p b c f -> p (b c f)"), AF.Exp)
            state[("E", p)] = E

        def emit_gate(p):
            # top-8 / den / rden / msk on DVE, processed in TWO
            # half-batches so the GPSIMD U-mults (which pace the Gt
            # matmuls two iterations later) start ~2.5us earlier.
            # For the LAST pair everything runs on DVE per-chunk so the
            # tail drain is latency-bound, not handoff-bound.
            last = (p == npairs - 1)
            E = state.pop(("E", p))
            top8 = sp.tile([128, 4, 8], f32, name="top8", tag="top8",
                           bufs=4)
            den = sp.tile([128, 4], f32, name="den", tag="den", bufs=4)
            rden = sp.tile([128, 4], f32, name="rden", tag="rden", bufs=4)
            Us = [sp.tile([128, 2, N], bf16, name="U", tag=f"U{bi}",
                          bufs=4) for bi in range(2)]
            for bi in range(2):
                for ic in range(2):
                    c = 2 * bi + ic
                    nc.vector.max(out=top8[:, c, :], in_=E[:, c, :])
                nc.vector.tensor_reduce(
                    out=den[:, 2 * bi:2 * bi + 2],
                    in_=top8[:, 2 * bi:2 * bi + 2, :],
                    axis=mybir.AxisListType.X, op=OP.add)
                nc.vector.reciprocal(rden[:, 2 * bi:2 * bi + 2],
                                     den[:, 2 * bi:2 * bi + 2])
                for ic in range(2):
                    c = 2 * bi + ic
                    msk = sp.tile([128, N], bf16, name="msk", tag="msk",
                                  bufs=8)
                    nc.vector.tensor_scalar(
                        out=msk[:], in0=E[:, c, :],
                        scalar1=top8[:, c, 7:8],
                        scalar2=rden[:, c:c + 1],
                        op0=OP.is_ge, op1=OP.mult)
                    eng = nc.vector if last else nc.gpsimd
                    eng.tensor_tensor(
                        out=Us[bi][:, ic, :], in0=msk[:],
                        in1=E[:, c, :], op=OP.mult)
            state[("gate", p)] = Us

        def emit_gt(p):
            # gate transpose via the DMA XBAR (2-byte hardware
            # transpose): replaces 8 PE matmuls + a [128,1024] scalar
            # cast + a PSUM ring with 4 DMA issues on the idle sync
            # queue. Out layout [j_lo, j_hi, i] matches msgs' chunk-
            # major j so the agg matmul consumes it directly.
            Us = state.pop(("gate", p))
            Gt_bs = []
            for bi in range(2):
                Gt_sb = sp.tile([128, 2, 2, 128], bf16, name="Gt_sb",
                                tag="Gt", bufs=2)
                for ic in range(2):
                    nc.sync.dma_start_transpose(
                        Gt_sb[:, :, ic, :], Us[bi][:, ic, :])
                Gt_bs.append(Gt_sb)
            state[("Gt", p)] = Gt_bs

        def emit_agg(p):
            Gt_b = state.pop(("Gt", p))
            msgs_b = state.pop(("msgs", p))
            aggT_ps = pp.tile([MSG, 2, N], f32, tag="e", bufs=1,
                              name="aggT_ps")
            for bi in range(2):
                nc.tensor.matmul(
                    aggT_ps[:, bi, :], msgs_b[:, 2 * bi, :],
                    Gt_b[bi][:, 0].rearrange("p c f -> p (c f)"),
                    start=True, stop=False)
                nc.tensor.matmul(
                    aggT_ps[:, bi, :], msgs_b[:, 2 * bi + 1, :],
                    Gt_b[bi][:, 1].rearrange("p c f -> p (c f)"),
                    start=False, stop=True)
            state[("aggT_ps", p)] = aggT_ps

        def emit_agg_cp(p):
            aggT_ps = state.pop(("aggT_ps", p))
            aggT_r = aggT_tiles[p % 2]
            nc.vector.tensor_copy(
                aggT_r[0:MSG, :, :].rearrange("m b n -> m (b n)"),
                aggT_ps[:].rearrange("m b n -> m (b n)"))

        def emit_l1(p):
            ob = state[("od", p)]
            aggT_r = aggT_tiles[p % 2]
            # last pair borrows the (dead by then) Gt ring so its l1
            # doesn't serialize behind relu_r(p-1) on the a-ring
            r_ps = pp.tile([128, 2, 2 * N], f32,
                           tag="c" if p == npairs - 1 else "a", bufs=1,
                           name="r_ps")
            aggT_ap = aggT_r[:].rearrange("m b n -> m (b n)")
            od0 = ob[:, 0].rearrange("d b n -> d (b n)")
            od1 = ob[:, 1].rearrange("d b n -> d (b n)")
            for mi in range(2):
                ms = 128 * mi
                nc.tensor.matmul(r_ps[:, mi, :], Wr1_r0[:, ms:ms + 128],
                                 od0, start=True, stop=False)
                nc.tensor.matmul(r_ps[:, mi, :], Wr1_r1[:, ms:ms + 128],
                                 od1, start=False, stop=False)
                nc.tensor.matmul(r_ps[:, mi, :], Wr1c_b[:, ms:ms + 128],
                                 aggT_ap, start=False, stop=True)
            state[("r_ps", p)] = r_ps

        def emit_relu_r(p):
            r_ps = state.pop(("r_ps", p))
            rT = sp.tile([128, 2, 2 * N], bf16, name="rT", tag="rT",
                         bufs=4)
            nc.scalar.activation(
                rT[:].rearrange("h c f -> h (c f)"),
                r_ps[:].rearrange("h c f -> h (c f)"), AF.Relu)
            state[("rT", p)] = rT

        def emit_l2(p):
            rT = state.pop(("rT", p))
            state.pop(("od", p))
            state.pop(("od8", p))
            o_ps = pp.tile([128, 2, 2 * N], f32,
                           tag="a" if p == npairs - 1 else "b", bufs=1,
                           name="o_ps")
            for dc in range(2):
                ds = 128 * dc
                nc.tensor.matmul(o_ps[:, dc, :], Wr2_r0[:, ds:ds + 128],
                                 rT[:, 0, :], start=True, stop=False)
                nc.tensor.matmul(o_ps[:, dc, :], Wr2_r1[:, ds:ds + 128],
                                 rT[:, 1, :], start=False, stop=True)
            o_sb = sp.tile([128, 2, 2, N], bf16, name="o_sb", tag="o_sb",
                           bufs=3)
            nc.scalar.activation(
                o_sb[:].rearrange("d c b n -> d (c b n)"),
                o_ps[:].rearrange("d c f -> d (c f)"), AF.Copy)
            nc.sync.dma_start(out_v[:, :, 2 * p:2 * p + 2, :], o_sb[:])

        # ---------------- main pipeline loop ----------------
        # Per-iteration emission order fixes each engine's queue order:
        #   PE:     hT(v) tmp(v) l2(v-4) msn(v) Gt(v-2) s(v) l1(v-3)
        #           aggT(v-2)
        #   scalar: relu_h(v) out(v-4) msn-cp(v) Gt-cast(v-2)
        #           relu_r(v-3) exp(v)
        #   vector: tmpT(v) Max8/den/msk(v-1) aggT-cp(v-2)
        #   gpsimd: U-mult(v-1) x4
        for _ in range(passes):
            emit_od(0)
            for v in range(npairs + 4):
                if v < npairs:
                    if v + 1 < npairs:
                        emit_od(v + 1)
                    if v >= 1:
                        emit_exp(v - 1)
                    emit_ht(v)
                    emit_tmp(v)
                    if v >= 4:
                        emit_l2(v - 4)
                    emit_msn(v)
                    emit_scores(v)
                    if v >= 2:
                        emit_gt(v - 2)
                    if v >= 3:
                        emit_l1(v - 3)
                        emit_relu_r(v - 3)
                    if v >= 1:
                        emit_gate(v - 1)
                    if v >= 2:
                        emit_agg(v - 2)
                        emit_agg_cp(v - 2)
                else:
                    # epilogue: latency-ordered drain
                    if npairs <= v < npairs + 1:
                        emit_exp(v - 1)
                    if 1 <= v < npairs + 1:
                        emit_gate(v - 1)
                    if 2 <= v < npairs + 2:
                        emit_gt(v - 2)
                    if 3 <= v < npairs + 3:
                        emit_l1(v - 3)
                        emit_relu_r(v - 3)
                    if 2 <= v < npairs + 2:
                        emit_agg(v - 2)
                        emit_agg_cp(v - 2)
                    if 4 <= v:
                        emit_l2(v - 4)

    nc.compile()
    return nc


def _np_inputs_for_core(inputs, core, bpc=BPC):
    import ml_dtypes

    bf = ml_dtypes.bfloat16
    f8 = ml_dtypes.float8_e4m3
    obs = np.asarray(inputs["obs_all"], np.float32)
    lo = core * bpc
    obsT = np.ascontiguousarray(
        obs[lo:lo + bpc].transpose(2, 0, 1))            # [D, bpc, N] f32
    obs8 = np.ascontiguousarray(
        obsT.reshape(2, 128, bpc, N).transpose(1, 0, 2, 3))

    W1 = np.asarray(inputs["W1"], np.float32)
    W2 = np.asarray(inputs["W2"], np.float32)
    b2 = np.asarray(inputs["b2"], np.float32)
    Wc = np.asarray(inputs["Wc"], np.float32)
    bc = np.asarray(inputs["bc"], np.float32)
    Wd = np.asarray(inputs["Wd"], np.float32)
    bd = np.asarray(inputs["bd"], np.float32)
    Wr1 = np.asarray(inputs["Wr1"], np.float32)
    br1 = np.asarray(inputs["br1"], np.float32)
    Wbil = np.asarray(inputs["Wbil"], np.float32)

    Wf = (W2 @ Wc) @ Wd                              # [H1, MSG]
    bf_vec = (b2 @ Wc) @ Wd + bc @ Wd + bd           # [MSG]
    Wr1c = Wr1[D:D + MSG]                            # [MSG, H2]
    # ones-row carries the folded message bias AND br1
    Wr1c_aug = np.vstack([Wr1c, (bf_vec @ Wr1c + br1)[None, :]])

    return {
        "obsT": obsT.astype(bf),
        "obs8": obs8.astype(f8),
        "W1": np.ascontiguousarray(
            W1.reshape(2, 128, H1).transpose(1, 0, 2)).astype(f8),
        "Wf": Wf.astype(bf),
        "Wbil": np.ascontiguousarray(
            Wbil.reshape(2, 128, D).transpose(1, 0, 2)).astype(f8),
        "Wr1a": Wr1[0:D].astype(bf),
        "Wr1c": np.ascontiguousarray(Wr1c_aug).astype(bf),
        "Wr2": np.asarray(inputs["Wr2"], np.float32).astype(bf),
        "b1": np.asarray(inputs["b1"], np.float32),
    }


def _finish(outT, br2):
    # outT: [D, bpc, N] bf16 -> [bpc, N, D] f32 + br2
    return outT.astype(np.float32).transpose(1, 2, 0) + br2[None, None, :]


def kernel(**inputs):
    from concourse.bass_utils import run_bass_kernel_spmd

    if "prog" not in _CACHE:
        _CACHE["prog"] = build_program(BPC)
    nc = _CACHE["prog"]

    br2 = np.asarray(inputs["br2"], np.float32)
    core_ids = list(range(NCORES))
    in_maps = [_np_inputs_for_core(inputs, c) for c in core_ids]
    res = run_bass_kernel_spmd(nc, in_maps, core_ids)
    out = np.concatenate(
        [_finish(np.asarray(res.results[c]["out"]), br2)
         for c in core_ids], axis=0)
    return out.astype(np.float32)


# revision 31
# speedup vs baseline: 1.5540x; 1.5540x over previous
"""Trainium2 Bass kernel for nn_BandwidthConstrainedComm.

GNN message passing: per batch element, N=256 agents each generate a
message (MLP -> compress -> decompress), compute pairwise bilinear
relevance scores, pick top-K=8 senders (softmax gated), aggregate their
messages, and run a receiver MLP over [obs, agg].

Sharding: pure data parallel over batch B=128 -> 16 per core x 8 cores.

Design notes (v4 - 5-stage pipeline + fp8 DoubleRow front-end):
  - obs uploaded twice: bf16 [d,2(dc),b,n] for the receiver MLP (fp8
    there breaks tolerance) and fp8-e4m3 [d_lo,2(d_hi),b,n] for the
    message/score path (verified: max-err unchanged, the bf16 receiver
    path dominates).
  - h-MLP, bilinear tmp and scores run as fp8 DoubleRow matmuls
    (K=256 per instruction, ~2x fewer PE instructions, 1.44x rate).
    tmpT is cast PSUM->fp8 so scores' stationary operand is fp8 too.
  - W2@Wc@Wd fused into one [H1, MSG] matrix on the host; message bias
    (+ br1) folded into the receiver matmul via a ones-row in aggT and
    an extra host-precomputed row in Wr1c; br2 added on the host.
  - top-8 via DVE Max8; den=sum(top8) one reduce; msk=(E>=t8)*rden on
    DVE (2-scalar tensor_scalar); U = msk*E on GPSIMD tensor_tensor
    (the only fast Pool op); gate transpose Gt = U.T @ I by PE.
  - 5-stage pipeline: pre(v) | gate(v-1) | Gt+agg(v-2) | l1(v-3) |
    l2+out(v-4), with per-engine queue orders chosen so every
    cross-engine dependency has >= half an iteration of slack.
  - engine budget per pair: PE ~4us, scalar relu_h/out/msn/Gt-cast/
    relu_r/exp ~5.5us, vector tmpT/Max8/den/recip/msk/aggT ~5.3us,
    gpsimd 4 mults ~2.7us.
  - PSUM 16KB/partition exactly: g[2K]=hT/msn, a[4K]=tmp/r, b[4K]=o/s,
    c[4K]=Gt, e[2K]=aggT.
  - output written as bf16 [D, bpc, N] in one DMA, un-transposed and
    f32-cast on the host.
"""

import sys

sys.path.insert(0, "/opt/trn_rl_repo")

import numpy as np

# problem dims (hardcoded per contract)
B, N, D = 128, 256, 256
MSG, CD, K = 64, 32, 8
H1, H2 = 128, 256
NCORES = 8
BPC = B // NCORES  # batches per core

_CACHE = {}


def build_program(bpc=BPC, passes=1):
    import concourse.bacc as bacc
    import concourse.mybir as mybir
    import concourse.tile as tile
    from concourse.masks import make_identity
    from contextlib import ExitStack

    dt = mybir.dt
    f32, bf16, f8 = dt.float32, dt.bfloat16, dt.float8e4
    AF = mybir.ActivationFunctionType
    OP = mybir.AluOpType
    DR = mybir.MatmulPerfMode.DoubleRow

    assert bpc % 2 == 0
    npairs = bpc // 2

    nc = bacc.Bacc("TRN2", target_bir_lowering=False, debug=False,
                   num_devices=NCORES)

    obsT_d = nc.dram_tensor("obsT", [D, bpc, N], bf16, kind="ExternalInput")
    obs8_d = nc.dram_tensor("obs8", [128, 2, bpc, N], f8,
                            kind="ExternalInput")
    W1_d = nc.dram_tensor("W1", [128, 2, H1], f8, kind="ExternalInput")
    Wf_d = nc.dram_tensor("Wf", [H1, MSG], bf16, kind="ExternalInput")
    Wbil_d = nc.dram_tensor("Wbil", [128, 2, D], f8, kind="ExternalInput")
    Wr1a_d = nc.dram_tensor("Wr1a", [D, H2], bf16, kind="ExternalInput")
    Wr1c_d = nc.dram_tensor("Wr1c", [MSG + 1, H2], bf16,
                            kind="ExternalInput")
    Wr2_d = nc.dram_tensor("Wr2", [H2, D], bf16, kind="ExternalInput")
    b1_d = nc.dram_tensor("b1", [H1], f32, kind="ExternalInput")
    out_d = nc.dram_tensor("out", [D, bpc, N], bf16, kind="ExternalOutput")

    with tile.TileContext(nc) as tc, ExitStack() as ctx:
        wp = ctx.enter_context(tc.tile_pool(name="wp", bufs=1))
        dp = ctx.enter_context(tc.tile_pool(name="dp", bufs=5))
        sp = ctx.enter_context(tc.tile_pool(name="sp", bufs=3))
        pp = ctx.enter_context(tc.tile_pool(name="pp", bufs=1, space="PSUM"))

        # ---------------- one-time setup ----------------
        ident = wp.tile([128, 128], f32)
        make_identity(nc, ident[:])
        ident_b = wp.tile([128, 128], bf16)
        nc.vector.tensor_copy(ident_b[:], ident[:])

        def loadw(dram_ap, shape, name, dtype=bf16, eng=nc.scalar):
            t = wp.tile(shape, dtype, name=name)
            eng.dma_start(t[:], dram_ap)
            return t

        W1_8 = loadw(W1_d[:], [128, 2, H1], "W1", f8)
        Wbil_8 = loadw(Wbil_d[:], [128, 2, D], "Wbil", f8, nc.gpsimd)
        Wf_b = loadw(Wf_d[:], [H1, MSG], "Wf")
        Wr1_r0 = loadw(Wr1a_d[0:128, :], [128, H2], "Wr1a", bf16, nc.gpsimd)
        Wr1_r1 = loadw(Wr1a_d[128:256, :], [128, H2], "Wr1b")
        Wr1c_b = loadw(Wr1c_d[:], [MSG + 1, H2], "Wr1c", bf16, nc.gpsimd)
        Wr2_r0 = loadw(Wr2_d[0:128, :], [128, D], "Wr2a")
        Wr2_r1 = loadw(Wr2_d[128:256, :], [128, D], "Wr2b", bf16, nc.gpsimd)

        b1_sb = wp.tile([H1, 1], f32, name="b1s")
        nc.scalar.dma_start(
            b1_sb[:], b1_d[:].rearrange("(p o) -> p o", o=1))

        # persistent aggT tiles with a constant ones-row (row MSG) for
        # the folded message bias (+ br1)
        aggT_tiles = []
        for i in range(2):
            t = wp.tile([MSG + 1, 2, N], bf16, name=f"aggTp{i}")
            nc.vector.memset(t[MSG:MSG + 1, :, :], 1.0)
            aggT_tiles.append(t)

        obsT_v = obsT_d[:].rearrange("(c d) b n -> d c b n", c=2)
        out_v = out_d[:].rearrange("(c d) b n -> d c b n", c=2)

        # ---------------- pipeline stages ----------------
        state = {}

        def emit_od(p):
            o8 = dp.tile([128, 2, 2, N], f8, name="od8", tag="od8", bufs=3)
            nc.sync.dma_start(o8[:], obs8_d[:, :, 2 * p:2 * p + 2, :])
            ob = dp.tile([128, 2, 2, N], bf16, name="od", tag="od", bufs=5)
            nc.sync.dma_start(ob[:], obsT_v[:, :, 2 * p:2 * p + 2, :])
            state[("od", p)] = ob
            state[("od8", p)] = o8

        def emit_ht(p):
            o8 = state[("od8", p)]
            hT_ps = pp.tile([H1, 2 * N], f32, tag="g", bufs=1)
            nc.tensor.matmul(hT_ps[:], W1_8[:],
                             o8[:].rearrange("d c b n -> d c (b n)"),
                             start=True, stop=True, perf_mode=DR)
            hT_b = sp.tile([H1, 2 * N], bf16, name="hT_b", tag="hT",
                           bufs=3)
            nc.scalar.activation(hT_b[:], hT_ps[:], AF.Relu, bias=b1_sb[:])
            state[("hT", p)] = hT_b

        def emit_tmp(p):
            o8 = state[("od8", p)]
            rhs = o8[:].rearrange("d c b n -> d c (b n)")
            tmp_ps = pp.tile([128, 2, 2 * N], f32, tag="a", bufs=1)
            for ec in range(2):
                nc.tensor.matmul(tmp_ps[:, ec, :],
                                 Wbil_8[:, :, 128 * ec:128 * (ec + 1)],
                                 rhs, start=True, stop=True, perf_mode=DR)
            tmpT8 = sp.tile([128, 2, 2 * N], f8, name="tmpT8",
                            tag="tmpT", bufs=3)
            nc.vector.tensor_copy(
                tmpT8[:].rearrange("e c f -> e (c f)"),
                tmp_ps[:].rearrange("e c f -> e (c f)"))
            state[("tmpT", p)] = tmpT8

        def emit_msn(p):
            hT_b = state.pop(("hT", p))
            msn_ps = pp.tile([128, 4, MSG], f32, tag="g", bufs=1,
                             name="msn_ps")
            for q in range(4):
                nc.tensor.matmul(msn_ps[:, q, :],
                                 hT_b[:, 128 * q:128 * (q + 1)],
                                 Wf_b[:], start=True, stop=True)
            msgs_b = sp.tile([128, 4, MSG], bf16, name="msgs_b",
                             tag="msgs", bufs=4)
            nc.scalar.activation(
                msgs_b[:].rearrange("p q m -> p (q m)"),
                msn_ps[:].rearrange("p q m -> p (q m)"), AF.Copy)
            state[("msgs", p)] = msgs_b

        def emit_scores(p):
            o8 = state[("od8", p)]
            tmpT8 = state.pop(("tmpT", p))
            s_ps = pp.tile([128, 2, 2, N], f32, tag="b", bufs=1,
                           name="s_ps")
            for bi in range(2):
                boff = bi * N
                for ic in range(2):
                    ioff = boff + 128 * ic
                    nc.tensor.matmul(s_ps[:, bi, ic, :],
                                     tmpT8[:, :, ioff:ioff + 128],
                                     o8[:, :, bi, :],
                                     start=True, stop=True, perf_mode=DR)
            state[("s_ps", p)] = s_ps

        def emit_exp(p):
            s_ps = state.pop(("s_ps", p))
            E = sp.tile([128, 4, N], bf16, name="E", tag="E", bufs=5)
            nc.scalar.activation(
                E[:].rearrange("p c f -> p (c f)"),
                s_ps[:].rearrange("p b c f -> p (b c f)"), AF.Exp)
            state[("E", p)] = E

        def emit_gate(p):
            # top-8 / den / rden / msk on DVE, processed in TWO
            # half-batches so the GPSIMD U-mults (which pace the Gt
            # matmuls two iterations later) start ~2.5us earlier.
            # For the LAST pair everything runs on DVE per-chunk so the
            # tail drain is latency-bound, not handoff-bound.
            last = (p == npairs - 1)
            E = state.pop(("E", p))
            top8 = sp.tile([128, 4, 8], f32, name="top8", tag="top8",
                           bufs=4)
            den = sp.tile([128, 4], f32, name="den", tag="den", bufs=4)
            rden = sp.tile([128, 4], f32, name="rden", tag="rden", bufs=4)
            Us = [sp.tile([128, 2, N], bf16, name="U", tag=f"U{bi}",
                          bufs=4) for bi in range(2)]
            for bi in range(2):
                for ic in range(2):
                    c = 2 * bi + ic
                    nc.vector.max(out=top8[:, c, :], in_=E[:, c, :])
                nc.vector.tensor_reduce(
                    out=den[:, 2 * bi:2 * bi + 2],
                    in_=top8[:, 2 * bi:2 * bi + 2, :],
                    axis=mybir.AxisListType.X, op=OP.add)
                nc.vector.reciprocal(rden[:, 2 * bi:2 * bi + 2],
                                     den[:, 2 * bi:2 * bi + 2])
                for ic in range(2):
                    c = 2 * bi + ic
                    msk = sp.tile([128, N], bf16, name="msk", tag="msk",
                                  bufs=8)
                    nc.vector.tensor_scalar(
                        out=msk[:], in0=E[:, c, :],
                        scalar1=top8[:, c, 7:8],
                        scalar2=rden[:, c:c + 1],
                        op0=OP.is_ge, op1=OP.mult)
                    eng = nc.vector if last else nc.gpsimd
                    eng.tensor_tensor(
                        out=Us[bi][:, ic, :], in0=msk[:],
                        in1=E[:, c, :], op=OP.mult)
            state[("gate", p)] = Us

        def emit_gt(p):
            Us = state.pop(("gate", p))
            Gt_ps = pp.tile([128, 2, 2, N], f32, tag="c", bufs=1,
                            name="Gt_ps")
            for bi in range(2):
                U = Us[bi]
                for ic in range(2):
                    for jc in range(2):
                        nc.tensor.matmul(
                            Gt_ps[:, bi, jc, 128 * ic:128 * (ic + 1)],
                            U[:, ic, 128 * jc:128 * (jc + 1)],
                            ident_b[:], start=True, stop=True)
            Gt_b = sp.tile([128, 2, 2, N], bf16, name="Gt_b", tag="Gt",
                           bufs=3)
            nc.scalar.activation(
                Gt_b[:].rearrange("p b c f -> p (b c f)"),
                Gt_ps[:].rearrange("p b c f -> p (b c f)"), AF.Copy)
            state[("Gt", p)] = Gt_b

        def emit_agg(p):
            Gt_b = state.pop(("Gt", p))
            msgs_b = state.pop(("msgs", p))
            aggT_ps = pp.tile([MSG, 2, N], f32, tag="e", bufs=1,
                              name="aggT_ps")
            for bi in range(2):
                nc.tensor.matmul(aggT_ps[:, bi, :],
                                 msgs_b[:, 2 * bi, :], Gt_b[:, bi, 0, :],
                                 start=True, stop=False)
                nc.tensor.matmul(aggT_ps[:, bi, :],
                                 msgs_b[:, 2 * bi + 1, :],
                                 Gt_b[:, bi, 1, :],
                                 start=False, stop=True)
            state[("aggT_ps", p)] = aggT_ps

        def emit_agg_cp(p):
            aggT_ps = state.pop(("aggT_ps", p))
            aggT_r = aggT_tiles[p % 2]
            nc.vector.tensor_copy(
                aggT_r[0:MSG, :, :].rearrange("m b n -> m (b n)"),
                aggT_ps[:].rearrange("m b n -> m (b n)"))

        def emit_l1(p):
            ob = state[("od", p)]
            aggT_r = aggT_tiles[p % 2]
            # last pair borrows the (dead by then) Gt ring so its l1
            # doesn't serialize behind relu_r(p-1) on the a-ring
            r_ps = pp.tile([128, 2, 2 * N], f32,
                           tag="c" if p == npairs - 1 else "a", bufs=1,
                           name="r_ps")
            aggT_ap = aggT_r[:].rearrange("m b n -> m (b n)")
            od0 = ob[:, 0].rearrange("d b n -> d (b n)")
            od1 = ob[:, 1].rearrange("d b n -> d (b n)")
            for mi in range(2):
                ms = 128 * mi
                nc.tensor.matmul(r_ps[:, mi, :], Wr1_r0[:, ms:ms + 128],
                                 od0, start=True, stop=False)
                nc.tensor.matmul(r_ps[:, mi, :], Wr1_r1[:, ms:ms + 128],
                                 od1, start=False, stop=False)
                nc.tensor.matmul(r_ps[:, mi, :], Wr1c_b[:, ms:ms + 128],
                                 aggT_ap, start=False, stop=True)
            state[("r_ps", p)] = r_ps

        def emit_relu_r(p):
            r_ps = state.pop(("r_ps", p))
            rT = sp.tile([128, 2, 2 * N], bf16, name="rT", tag="rT",
                         bufs=4)
            nc.scalar.activation(
                rT[:].rearrange("h c f -> h (c f)"),
                r_ps[:].rearrange("h c f -> h (c f)"), AF.Relu)
            state[("rT", p)] = rT

        def emit_l2(p):
            rT = state.pop(("rT", p))
            state.pop(("od", p))
            state.pop(("od8", p))
            o_ps = pp.tile([128, 2, 2 * N], f32,
                           tag="a" if p == npairs - 1 else "b", bufs=1,
                           name="o_ps")
            for dc in range(2):
                ds = 128 * dc
                nc.tensor.matmul(o_ps[:, dc, :], Wr2_r0[:, ds:ds + 128],
                                 rT[:, 0, :], start=True, stop=False)
                nc.tensor.matmul(o_ps[:, dc, :], Wr2_r1[:, ds:ds + 128],
                                 rT[:, 1, :], start=False, stop=True)
            o_sb = sp.tile([128, 2, 2, N], bf16, name="o_sb", tag="o_sb",
                           bufs=3)
            nc.scalar.activation(
                o_sb[:].rearrange("d c b n -> d (c b n)"),
                o_ps[:].rearrange("d c f -> d (c f)"), AF.Copy)
            nc.sync.dma_start(out_v[:, :, 2 * p:2 * p + 2, :], o_sb[:])

        # ---------------- main pipeline loop ----------------
        # Per-iteration emission order fixes each engine's queue order:
        #   PE:     hT(v) tmp(v) l2(v-4) msn(v) Gt(v-2) s(v) l1(v-3)
        #           aggT(v-2)
        #   scalar: relu_h(v) out(v-4) msn-cp(v) Gt-cast(v-2)
        #           relu_r(v-3) exp(v)
        #   vector: tmpT(v) Max8/den/msk(v-1) aggT-cp(v-2)
        #   gpsimd: U-mult(v-1) x4
        for _ in range(passes):
            emit_od(0)
            for v in range(npairs + 4):
                if v < npairs:
                    if v + 1 < npairs:
                        emit_od(v + 1)
                    if v >= 1:
                        emit_exp(v - 1)
                    emit_ht(v)
                    emit_tmp(v)
                    if v >= 4:
                        emit_l2(v - 4)
                    emit_msn(v)
                    emit_scores(v)
                    if v >= 2:
                        emit_gt(v - 2)
                    if v >= 3:
                        emit_l1(v - 3)
                        emit_relu_r(v - 3)
                    if v >= 1:
                        emit_gate(v - 1)
                    if v >= 2:
                        emit_agg(v - 2)
                        emit_agg_cp(v - 2)
                else:
                    # epilogue: latency-ordered drain
                    if npairs <= v < npairs + 1:
                        emit_exp(v - 1)
                    if 1 <= v < npairs + 1:
                        emit_gate(v - 1)
                    if 2 <= v < npairs + 2:
                        emit_gt(v - 2)
                    if 3 <= v < npairs + 3:
                        emit_l1(v - 3)
                        emit_relu_r(v - 3)
                    if 2 <= v < npairs + 2:
                        emit_agg(v - 2)
                        emit_agg_cp(v - 2)
                    if 4 <= v:
                        emit_l2(v - 4)

    nc.compile()
    return nc


def _np_inputs_for_core(inputs, core, bpc=BPC):
    import ml_dtypes

    bf = ml_dtypes.bfloat16
    f8 = ml_dtypes.float8_e4m3
    obs = np.asarray(inputs["obs_all"], np.float32)
    lo = core * bpc
    obsT = np.ascontiguousarray(
        obs[lo:lo + bpc].transpose(2, 0, 1))            # [D, bpc, N] f32
    obs8 = np.ascontiguousarray(
        obsT.reshape(2, 128, bpc, N).transpose(1, 0, 2, 3))

    W1 = np.asarray(inputs["W1"], np.float32)
    W2 = np.asarray(inputs["W2"], np.float32)
    b2 = np.asarray(inputs["b2"], np.float32)
    Wc = np.asarray(inputs["Wc"], np.float32)
    bc = np.asarray(inputs["bc"], np.float32)
    Wd = np.asarray(inputs["Wd"], np.float32)
    bd = np.asarray(inputs["bd"], np.float32)
    Wr1 = np.asarray(inputs["Wr1"], np.float32)
    br1 = np.asarray(inputs["br1"], np.float32)
    Wbil = np.asarray(inputs["Wbil"], np.float32)

    Wf = (W2 @ Wc) @ Wd                              # [H1, MSG]
    bf_vec = (b2 @ Wc) @ Wd + bc @ Wd + bd           # [MSG]
    Wr1c = Wr1[D:D + MSG]                            # [MSG, H2]
    # ones-row carries the folded message bias AND br1
    Wr1c_aug = np.vstack([Wr1c, (bf_vec @ Wr1c + br1)[None, :]])

    return {
        "obsT": obsT.astype(bf),
        "obs8": obs8.astype(f8),
        "W1": np.ascontiguousarray(
            W1.reshape(2, 128, H1).transpose(1, 0, 2)).astype(f8),
        "Wf": Wf.astype(bf),
        "Wbil": np.ascontiguousarray(
            Wbil.reshape(2, 128, D).transpose(1, 0, 2)).astype(f8),
        "Wr1a": Wr1[0:D].astype(bf),
        "Wr1c": np.ascontiguousarray(Wr1c_aug).astype(bf),
        "Wr2": np.asarray(inputs["Wr2"], np.float32).astype(bf),
        "b1": np.asarray(inputs["b1"], np.float32),
    }


def _finish(outT, br2):
    # outT: [D, bpc, N] bf16 -> [bpc, N, D] f32 + br2
    return outT.astype(np.float32).transpose(1, 2, 0) + br2[None, None, :]


def kernel(**inputs):
    from concourse.bass_utils import run_bass_kernel_spmd

    if "prog" not in _CACHE:
        _CACHE["prog"] = build_program(BPC)
    nc = _CACHE["prog"]

    br2 = np.asarray(inputs["br2"], np.float32)
    core_ids = list(range(NCORES))
    in_maps = [_np_inputs_for_core(inputs, c) for c in core_ids]
    res = run_bass_kernel_spmd(nc, in_maps, core_ids)
    out = np.concatenate(
        [_finish(np.asarray(res.results[c]["out"]), br2)
         for c in core_ids], axis=0)
    return out.astype(np.float32)
